# revision 11
# baseline (speedup 1.0000x reference)
"""Trainium2 Bass kernel (v5) for nn_Attn: batched column-softmax attention energies.

Math (per batch element b):
    E = encoder_outputs[:, b, :]            # [H, T]
    d = decoder_hidden[b]                   # [H]
    s = E^T d                               # [T]  (scores)
    w[h, t] = E[h, t] * s[t]
    sm = softmax over h of w (per column t)
    out[b, h] = sum_t sm[h, t]

Design (per core, data parallel over batch: 8 cores x 8 batch elements):
    - E loaded via gpsimd (SWDGE) casting DMA as float16: rounding at DMA time
      is free. HW-measured end-to-end rel err 1.0e-2 vs the 2e-2 gate (bf16 E
      fails at 3.8e-2; fp16 passes, matching a numpy bit-accurate prediction).
    - E transposes as fp16 transpose-mode matmuls: 1.0 cyc/row (vs 2.0 fp32),
      and each Et PSUM tile is one bank instead of two (pp_bufs=3 deepens
      the transpose pipeline).
    - scores on PE instead of a DVE pass: sneg_row[1,T] = sum_i (-d_i)^T @ E_i
      (fp16, 1 cyc/row) per T-half, then tiny fp32 PE transposes produce
      per-partition score columns scT[128, NTC]. Removes v1's first full DVE
      pass per chunk AND the 4MB dbcast broadcast DMA.
    - DVE: ONE fused pass per chunk: junk(fp16) = -s*Et, accum min -> mneg =
      -max_h(s*Et). fp16 Et (PSUM) in + fp16 junk out = all-2-byte operands
      -> DVE 2x_1p mode, halving the pass (HW rel err 1.0e-2, 2x margin).
      Plus reciprocal 1/z + bf16 cast (NOT on Pool: gpsimd per-op launch is
      ~1.5us on real HW and sits on the critical path) and the s-score
      PSUM->SBUF copies.
    - ACT: e = exp(-junk + mneg) = exp(s*Et - max) as bf16, accum z (z >= 1
      so 1/z never divides by zero); output-row PSUM->SBUF copies stay on
      ACT (a DVE copy from PSUM partition-base 32 mis-executes on real HW
      despite passing CoreSim -- rel err 0.24 vs 1.0e-2).
    - output accumulates in ONE PSUM bank as two [1,512] halves on partitions
      {0, 32} (matmul tile_position allows base partition only in
      {0,32,64,96}), double buffered -> no inter-b pipeline bubble.
    - back-half ops (recip/cast/final matmuls) are emitted 2 chunks late so
      the in-order PE/DVE queues never stall on the softmax chain.
    - engine busy (cost model, per core): DMA ~90us, ACT ~88us, PE ~85us,
      DVE ~55us; predicted total 107us vs 187us for the v1 baseline.
      Best measured (same run_timed methodology as the 528us baseline):
      327-376us amid heavy shared-device noise.
"""

import numpy as np

import concourse.bass as bass
import concourse.mybir as mybir
from concourse.bass_utils import run_bass_kernel_spmd
from concourse.tile import TileContext

H = 1024
B = 64
T = 1024
N_CORES = 8
B_LOC = B // N_CORES  # 8 batch elements per core
NHC = H // 128        # 8 h-chunks
NTC = T // 128        # 8 t-chunks

F32 = mybir.dt.float32
F32R = mybir.dt.float32r
F16 = mybir.dt.float16
BF16 = mybir.dt.bfloat16


def _split_waits(nc, max_waits=1):
    """Workaround for this container's walrus: instructions accept only one
    sync-wait; hoist extra waits onto single-wait Drain carriers."""
    n_new = 0
    for f in nc.m.functions:
        for blk in f.blocks:
            new_insts = []
            for inst in blk.instructions:
                si = inst.sync_info
                if si is not None and si.on_wait is not None and len(si.on_wait) > max_waits:
                    waits = list(si.on_wait)
                    while len(waits) > max_waits:
                        w = waits.pop(0)
                        d = mybir.InstDrain(
                            name=f"I-ws-{nc.next_id()}", ins=[], outs=[]
                        )
                        d.engine = inst.engine
                        d.sync_info = mybir.SyncInfo(on_wait=[w], on_update=[])
                        new_insts.append(d)
                        n_new += 1
                    si.on_wait = waits
                new_insts.append(inst)
            blk.instructions = new_insts
    return n_new


def build_program(
    f32r=True,           # f32r natt (casting DMA) + f32r transposes + f32r s-matmuls
    f16=True,            # fp16 natt (1 cyc/row transposes, 1-bank Et PSUM)
    score_pe=True,       # scores via PE (else DVE pass like v1)
    o_compact=True,      # out accum as [2,512] in one PSUM bank, double buffered
    split_nat_dma=2,     # natt loaded in this many DMAs (pipeline head start; 2 aligns with the score-row halves and halves SWDGE descriptor count vs 4)
    rl_engine="vector",
    lag=2,               # chunks of emission lag for recip/cast/final-matmul
    nat_bufs=3,
    pp_bufs=3,
    w16=True,            # fp16 junk (-s*Et scratch): 2-byte in+out => DVE 2x_1p mode
    s2_engine="vector",   # engine for s2 PSUM->SBUF copies: scalar|vector
    orow_engine="scalar", # engine for o PSUM->SBUF copies: scalar|vector (vector mis-executes the partition-32 copy on real HW)
    junk_bufs=3,
    e_bufs=4,
    small_bufs=8,
    split_waits=True,
):
    nc = bass.Bass("TRN2", debug=False, num_devices=N_CORES)
    enc_h = nc.dram_tensor("enc", [H, B_LOC, T], F32, kind="ExternalInput")
    dec_h = nc.dram_tensor("dec", [B_LOC, H], F32, kind="ExternalInput")
    ident_h = nc.dram_tensor("ident", [128, 128], F32, kind="ExternalInput")
    out_h = nc.dram_tensor("out", [B_LOC, H], F32, kind="ExternalOutput")

    enc = enc_h.ap()
    dec = dec_h.ap()
    ident = ident_h.ap()
    out = out_h.ap()

    AF = mybir.ActivationFunctionType
    OP = mybir.AluOpType

    if f16:
        f32r = False
    NAT_DT = F16 if f16 else (F32R if f32r else F32)
    PP_DT = F16 if f16 else F32

    def mm_trans(out_ap, lhsT, rhs, **kw):
        if f32r:
            out_ap = out_ap.bitcast(F32R)
        nc.tensor.matmul(out_ap, lhsT=lhsT, rhs=rhs, is_transpose=True, **kw)

    with TileContext(nc) as tc:
        with (
            tc.tile_pool(name="const", bufs=1) as constp,
            tc.tile_pool(name="natp", bufs=nat_bufs) as natp,
            tc.tile_pool(name="junkp", bufs=junk_bufs) as junkp,
            tc.tile_pool(name="ep", bufs=e_bufs) as ep,
            tc.tile_pool(name="srowp", bufs=2) as srowp,
            tc.tile_pool(name="sctp", bufs=2) as sctp,
            tc.tile_pool(name="smallp", bufs=small_bufs) as smallp,
            tc.tile_pool(name="rowp", bufs=2) as rowp,
            tc.tile_pool(name="dbp", bufs=2) as dbp,  # only if not score_pe
            tc.tile_pool(name="ps_p", bufs=pp_bufs, space="PSUM") as ps_p,
            tc.tile_pool(name="ps_o", bufs=2 if o_compact else 1, space="PSUM") as ps_o,
            tc.tile_pool(name="ps_s", bufs=2, space="PSUM") as ps_s,   # 2 banks
        ):
            identsb = constp.tile([128, 128], F32, name="identsb")
            nc.sync.dma_start(out=identsb[:, :], in_=ident)
            if f32r or f16:
                identr = constp.tile([128, 128], NAT_DT, name="identr")
                nc.vector.tensor_scalar(identr[:, :], identsb[:, :], 1.0, None, OP.mult)
            else:
                identr = identsb

            decTn_sb = None
            if score_pe:
                # dec natural [B_LOC, H] (one contiguous 32KB DMA)
                dec_nat = constp.tile([B_LOC, H], F32, name="dec_nat")
                nc.sync.dma_start(out=dec_nat[:, :], in_=dec)
                # decT[p, i, b] = d[b, 128i+p]; negate while copying to SBUF
                decT_ps = ps_s.tile([128, NHC, B_LOC], F32, name="decT_ps", tag="ps_s")
                for i in range(NHC):
                    nc.tensor.matmul(
                        decT_ps[:, i, :],
                        lhsT=dec_nat[:, 128 * i : 128 * (i + 1)],
                        rhs=identsb[0:B_LOC, 0:B_LOC],
                        is_transpose=True,
                    )
                decTn_sb = constp.tile([128, NHC, B_LOC], NAT_DT, name="decTn_sb")
                nc.vector.tensor_scalar(
                    decTn_sb[:, :, :], decT_ps[:, :, :], -1.0, None, OP.mult
                )

            for b in range(B_LOC):
                # ---- natural-layout load: natt[p, i, t] = E[128*i + p, t]
                # (f32r: gpsimd SWDGE casting DMA rounds fp32 -> f32r in flight)
                enc_b = enc[:, b, :].rearrange("(ii p) t -> p ii t", p=128)
                natt = natp.tile([128, NHC, T], NAT_DT, name="natt", tag="nat")
                tw = T // split_nat_dma
                for q in range(split_nat_dma):
                    tsl = slice(q * tw, (q + 1) * tw)
                    if f32r or f16:
                        nc.gpsimd.dma_start(out=natt[:, :, tsl], in_=enc_b[:, :, tsl])
                    else:
                        nc.sync.dma_start(out=natt[:, :, tsl], in_=enc_b[:, :, tsl])

                if score_pe:
                    # ---- sneg_row[1, T] = sum_i (-d_i)^T @ E_i, per T-half so
                    # chunks j<4 only wait on natt's first half (pipeline head)
                    # (kept fp32: the [1,128]->[128,1] transposes below violate
                    # walrus's fp32r ISA restrictions in f32r form)
                    srow_sb = srowp.tile([1, T], F32, name="srow_sb", tag="srow")
                    scTn_sb = sctp.tile([128, NTC], F32, name="scTn_sb", tag="sct")
                    for half in range(2):
                        tsl = slice(512 * half, 512 * half + 512)
                        s2 = ps_s.tile([1, 512], F32, name="s2", tag="ps_s")
                        for i in range(NHC):
                            nc.tensor.matmul(
                                s2[0:1, :],
                                lhsT=decTn_sb[:, i, b : b + 1],
                                rhs=natt[:, i, tsl],
                                start=(i == 0),
                                stop=(i == NHC - 1),
                            )
                        if s2_engine == "vector":
                            nc.vector.tensor_scalar(srow_sb[0:1, tsl], s2[0:1, :], 1.0, None, OP.mult)
                        else:
                            nc.scalar.copy(srow_sb[0:1, tsl], s2[0:1, :])
                        # transpose this half's scores to columns:
                        # scT[t_p, j] = sneg[128j + t_p]
                        scT_ps = ps_s.tile([128, NTC // 2], F32, name="scT_ps", tag="ps_s")
                        for jj in range(NTC // 2):
                            j = half * (NTC // 2) + jj
                            nc.tensor.matmul(
                                scT_ps[:, jj : jj + 1],
                                lhsT=srow_sb[0:1, 128 * j : 128 * (j + 1)],
                                rhs=identsb[0:1, 0:1],
                                is_transpose=True,
                            )
                        nc.vector.tensor_scalar(
                            scTn_sb[:, half * (NTC // 2) : (half + 1) * (NTC // 2)],
                            scT_ps[:, :], 1.0, None, OP.mult,
                        )
                else:
                    dbcast = dbp.tile([128, H], F32, name="dbcast", tag="dbcast")
                    nc.sync.dma_start(
                        out=dbcast[:, :], in_=dec[b : b + 1, :].to_broadcast([128, H])
                    )

                # ---- per t-chunk: transpose, softmax, accumulate output
                if o_compact:
                    o_ps = ps_o.tile([128, 512], F32, name="o_ps", tag="ps_o")
                else:
                    o_ps = ps_o.tile([1, H], F32, name="o_ps", tag="ps_o")
                # Per chunk j: [transposes (PE), fused mult+min (DVE), exp (ACT)]
                # emitted at step j; [recip, bf16 cast (DVE), 2 final matmuls
                # (PE)] emitted `lag` chunks later so the in-order PE/DVE
                # queues never stall waiting for the softmax chain of the
                # current chunk.
                pend = {}
                for step in range(NTC + lag):
                    if step < NTC:
                        j = step
                        p_ps = ps_p.tile([128, H], PP_DT, name="p_ps", tag="ps_p")
                        for i in range(NHC):
                            mm_trans(
                                p_ps[:, 128 * i : 128 * (i + 1)],
                                lhsT=natt[:, i, 128 * j : 128 * (j + 1)],
                                rhs=identr[:, :],
                                start=(i % 4 == 0),
                                stop=(i % 4 == 3),
                            )
                        junk = junkp.tile([128, H], F16 if (w16 and f16) else F32,
                                          name="junk", tag="junk")
                        mneg = smallp.tile([128, 1], F32, name="mneg", tag="mneg")
                        if score_pe:
                            nc.vector.tensor_scalar(
                                junk[:, :],
                                p_ps[:, :] if f16 else p_ps[:, :].bitcast(F32),
                                scTn_sb[:, j : j + 1],
                                60000.0 if (w16 and f16) else 3.0e38,
                                OP.mult,
                                OP.min,
                                accum_out=mneg[:, :],
                            )
                        else:
                            s_neg = smallp.tile([128, 1], F32, name="s_neg", tag="s_neg")
                            nc.vector.scalar_tensor_tensor(
                                out=junk[:, :],
                                in0=p_ps[:, :] if f16 else p_ps[:, :].bitcast(F32),
                                scalar=-1.0,
                                in1=dbcast[:, :],
                                op0=OP.mult,
                                op1=OP.mult,
                                accum_out=s_neg[:, :],
                            )
                            junk2 = junkp.tile([128, H], F32, name="junk2", tag="junk")
                            nc.vector.tensor_scalar(
                                junk2[:, :],
                                p_ps[:, :] if f16 else p_ps[:, :].bitcast(F32),
                                s_neg[:, :],
                                3.0e38,
                                OP.mult,
                                OP.min,
                                accum_out=mneg[:, :],
                            )
                            junk = junk2

                        # e = exp(s*Et - max), z = sum_h e  (z >= 1)
                        e = ep.tile([128, H], BF16, name="e", tag="e")
                        z = smallp.tile([128, 1], F32, name="z", tag="z")
                        nc.scalar.activation(
                            e[:, :],
                            junk[:, :],
                            AF.Exp,
                            bias=mneg[:, :],
                            scale=-1.0,
                            accum_out=z[:, :],
                        )
                        pend[j] = (e, z)
                    if step >= lag:
                        k = step - lag
                        e, z = pend.pop(k)
                        r = smallp.tile([128, 1], F32, name="r", tag="r")
                        nc.vector.reciprocal(r[:, :], z[:, :])
                        rl = smallp.tile([128, 1], BF16, name="rl", tag="rl")
                        if rl_engine == "vector":
                            nc.vector.tensor_scalar(rl[:, :], r[:, :], 1.0, None, OP.mult)
                        elif rl_engine == "gpsimd":
                            nc.gpsimd.tensor_scalar(rl[:, :], r[:, :], 1.0, None, OP.mult)
                        else:
                            nc.scalar.copy(rl[:, :], r[:, :])
                        # out[h] += sum_t r_t * e[t, h]; halves at partitions {0, 32}
                        for half in range(2):
                            if o_compact:
                                o_slice = o_ps[32 * half : 32 * half + 1, :]
                            else:
                                o_slice = o_ps[0:1, 512 * half : 512 * half + 512]
                            nc.tensor.matmul(
                                o_slice,
                                lhsT=rl[:, :],
                                rhs=e[:, 512 * half : 512 * half + 512],
                                start=(k == 0),
                                stop=(k == NTC - 1),
                            )

                if o_compact:
                    orow = rowp.tile([33, 512], F32, name="orow", tag="orow")
                    if orow_engine == "vector":
                        nc.vector.tensor_scalar(orow[0:1, :], o_ps[0:1, :], 1.0, None, OP.mult)
                        nc.vector.tensor_scalar(orow[32:33, :], o_ps[32:33, :], 1.0, None, OP.mult)
                    else:
                        nc.scalar.copy(orow[0:1, :], o_ps[0:1, :])
                        nc.scalar.copy(orow[32:33, :], o_ps[32:33, :])
                    out_b = out[b : b + 1, :].rearrange("o (p f) -> (o p) f", p=2)
                    nc.sync.dma_start(out=out_b, in_=orow[0:33:32, :])
                else:
                    orow = rowp.tile([1, H], F32, name="orow", tag="orow")
                    nc.scalar.copy(orow[:, :], o_ps[0:1, :])
                    nc.sync.dma_start(out=out[b : b + 1, :], in_=orow[:, :])

    if split_waits:
        _split_waits(nc)
    return nc


def make_in_maps(decoder_hidden, encoder_outputs):
    dec = np.ascontiguousarray(np.asarray(decoder_hidden, dtype=np.float32))
    enc = np.ascontiguousarray(np.asarray(encoder_outputs, dtype=np.float32))
    assert dec.shape == (B, H) and enc.shape == (H, B, T)
    ident = np.eye(128, dtype=np.float32)
    in_maps = []
    for k in range(N_CORES):
        bsl = slice(k * B_LOC, (k + 1) * B_LOC)
        in_maps.append(
            {
                "enc": np.ascontiguousarray(enc[:, bsl, :]),
                "dec": np.ascontiguousarray(dec[bsl, :]),
                "ident": ident,
            }
        )
    return in_maps


_PROGRAM = None


def kernel(**inputs) -> np.ndarray:
    global _PROGRAM
    if _PROGRAM is None:
        _PROGRAM = build_program()
    in_maps = make_in_maps(inputs["decoder_hidden"], inputs["encoder_outputs"])
    res = run_bass_kernel_spmd(_PROGRAM, in_maps, core_ids=list(range(N_CORES)))
    return np.concatenate([r["out"] for r in res.results], axis=0)
